# revision 1
# baseline (speedup 1.0000x reference)
"""ConnectionProductBlock on 8 TRN2 NeuronCores.

out[b, c*K + k, h, w] = am_out[b, c, h, w] * first_out[b, k, h, w]
  with B=16, C=8, K=64, H=W=56.

Strategy (data parallel over batch, 2 batches per core, no communication):
  - SBUF layout puts channels on partitions, hw (=3136) on the free dim so
    every DMA moves long contiguous runs (12.5KB per partition).
  - first_out for the core's 2 batches loads once as [128, 3136]
    (partition = b*64 + k).
  - am needs a partition-broadcast (am[b, c] replicated across the 64 k
    partitions of batch b). Compute engines have fixed lane<->partition
    wiring, so the replication is done on the idle TensorEngine: a K=2
    selector matmul sel.T @ am[{b0,b1}, c] writes rep[p, f] = am[p//64, c, f]
    into PSUM in 448-column chunks.
  - VectorEngine multiplies first * rep into an SBUF staging tile per c,
    which is DMAed out as one 1.6MB transfer.
HBM traffic per core is the 14.6MB minimum -> memory-roofline bound.
"""

import numpy as np

B, C, K, H, W = 16, 8, 64, 56, 56
HW = H * W  # 3136
NCORES = 8
BPC = B // NCORES  # batches per core = 2
CHUNK = 448  # 3136 = 7 * 448; one PSUM bank holds 448 fp32 comfortably
NCHUNK = HW // CHUNK
NPLANE = 3  # bf16 planes per fp32 am value (hi/mid/lo)

_PROGRAMS = {}


def _build_program(
    repeat=1,
    do_compute=True,
    do_out_dma=True,
    dual_ring=True,
    do_pe=True,
    do_mul=True,
    mul_src="psum",
):
    """repeat>1 wraps the whole body in a hardware loop; used only by the
    local benchmark harness to amortize dispatch overhead when timing.
    do_compute/do_out_dma isolate pipeline components for benchmarking."""
    import contextlib

    import concourse.bacc as bacc
    import concourse.mybir as mybir
    import concourse.tile as tile

    nc = bacc.Bacc("TRN2", debug=False)
    # am, host-decomposed into 3 bf16 planes (hi/mid/lo Dekker split — their
    # sum reconstructs fp32 am to <=1 ulp), with the per-c selector blocks
    # appended on the free dim. Partition = plane*16 + b*8 + c. One DMA covers
    # data + selectors, so each matmul carries a single sem wait (the Matmult
    # instruction struct only has one sync-wait slot). bf16 matmuls stream
    # ~3x faster than fp32 and K=48 costs the same as K=16 (cost is N cycles).
    amsel = nc.dram_tensor(
        "amsel",
        [NPLANE * BPC * C, HW + C * BPC * K],
        mybir.dt.bfloat16,
        kind="ExternalInput",
    )
    first = nc.dram_tensor(
        "first", [BPC, K, HW], mybir.dt.float32, kind="ExternalInput"
    )
    out = nc.dram_tensor(
        "out", [BPC, C * K, HW], mybir.dt.float32, kind="ExternalOutput"
    )

    with tile.TileContext(nc) as tc:
        with (
            tc.tile_pool(name="ins", bufs=1) as ins_pool,
            tc.tile_pool(name="rep", bufs=8, space="PSUM") as psum_pool,
            tc.tile_pool(name="outs", bufs=3) as out_pool,
            tc.For_i(0, repeat, 1) if repeat > 1 else contextlib.nullcontext(),
        ):
            # first2[p] = first[p // 64, p % 64]  (both batches stacked)
            first2 = ins_pool.tile([BPC * K, HW], mybir.dt.float32)
            nc.sync.dma_start(
                out=first2[:], in_=first.ap().rearrange("b k f -> (b k) f")
            )
            # am3[(plane, b, c), :HW] = bf16 plane of am[b, c];
            # am3[:, HW + c*128 : HW + (c+1)*128] = selector block for c.
            # sel_c.T @ am3 accumulates the 3 planes in fp32 PSUM:
            # rep[p, f] = am[p // 64, c, f] — block-broadcast of channel c of
            # each batch across that batch's 64 k-partitions. (PE requires rhs
            # base partition in {0, 32, 64}, so the selector — not a strided
            # rhs view — encodes the channel pick.)
            am3 = ins_pool.tile(
                [NPLANE * BPC * C, HW + C * BPC * K], mybir.dt.bfloat16
            )
            nc.sync.dma_start(out=am3[:], in_=amsel.ap())

            out_ap = out.ap()
            for c in range(C):
                out_t = out_pool.tile([BPC * K, HW], mybir.dt.float32, tag="out")
                if not do_compute:
                    # bench-only: mark the tile written so sim allows the DMA
                    nc.vector.memset(out_t[:, 0:2], 0.0)
                if do_compute:
                    for j in range(NCHUNK):
                        f0 = j * CHUNK
                        rep = None
                        if do_pe:
                            rep = psum_pool.tile(
                                [BPC * K, CHUNK], mybir.dt.float32, tag="rep"
                            )
                            nc.tensor.matmul(
                                rep[:],
                                lhsT=am3[
                                    :, HW + c * BPC * K : HW + (c + 1) * BPC * K
                                ],
                                rhs=am3[:, f0 : f0 + CHUNK],
                                start=True,
                                stop=True,
                            )
                        if do_mul:
                            in1 = (
                                rep[:]
                                if (mul_src == "psum" and rep is not None)
                                else first2[:, f0 : f0 + CHUNK]
                            )
                            nc.vector.tensor_mul(
                                out_t[:, f0 : f0 + CHUNK],
                                first2[:, f0 : f0 + CHUNK],
                                in1,
                            )
                        elif do_pe:
                            pass
                    if not do_mul:
                        nc.vector.memset(out_t[:, 0:2], 0.0)
                if do_out_dma:
                    # One DMA per batch ([64, HW] each, contiguous in DRAM).
                    # b=0 on the SP HWDGE ring, b=1 on the ACT ring — the two
                    # rings run concurrently so both partition halves are in
                    # flight and all 16 SBUF ports stay busy.
                    engs = (nc.sync, nc.scalar) if dual_ring else (nc.sync, nc.sync)
                    for b, eng in ((0, engs[0]), (1, engs[1])):
                        eng.dma_start(
                            out=out_ap[b, c * K : (c + 1) * K, :],
                            in_=out_t[b * K : (b + 1) * K, :],
                        )
    nc.compile()
    return nc


def _get_program(repeat=1, **variant):
    key = (repeat, tuple(sorted(variant.items())))
    if key not in _PROGRAMS:
        _PROGRAMS[key] = _build_program(repeat, **variant)
    return _PROGRAMS[key]


def _make_sel():
    # One [16, 128] selector block per c, identical for every plane:
    # sel[b*C + c, c*128 + b*64 + k] = 1
    sel = np.zeros((BPC * C, C * BPC * K), dtype=np.float32)
    for c in range(C):
        for b in range(BPC):
            sel[b * C + c, c * BPC * K + b * K : c * BPC * K + (b + 1) * K] = 1.0
    return sel


def _make_amsel(am_core):
    """am_core [BPC*C, HW] fp32 -> [NPLANE*BPC*C, HW + 1024] bf16 with the
    hi/mid/lo Dekker planes stacked plane-major and selector blocks appended.
    hi + mid + lo == am exactly up to <=1 fp32 ulp."""
    import ml_dtypes

    bf16 = ml_dtypes.bfloat16
    planes = []
    r = am_core
    for _ in range(NPLANE):
        p = r.astype(bf16)
        r = r - p.astype(np.float32)
        planes.append(p)
    sel = _make_sel().astype(bf16)
    rows = [np.concatenate([p, sel], axis=1) for p in planes]
    return np.ascontiguousarray(np.concatenate(rows, axis=0))


def _run(am_np, first_np, **spmd_kwargs):
    from concourse.bass_utils import run_bass_kernel_spmd

    nc = _get_program()
    in_maps = []
    for i in range(NCORES):
        am_i = am_np[BPC * i : BPC * (i + 1)].reshape(BPC * C, HW)
        in_maps.append(
            {
                "amsel": _make_amsel(am_i),
                "first": np.ascontiguousarray(first_np[BPC * i : BPC * (i + 1)]),
            }
        )
    return run_bass_kernel_spmd(nc, in_maps, core_ids=list(range(NCORES)), **spmd_kwargs)


def kernel(am_out, first_out):
    am_np = np.asarray(am_out, dtype=np.float32).reshape(B, C, HW)
    first_np = np.asarray(first_out, dtype=np.float32).reshape(B, K, HW)
    res = _run(am_np, first_np)
    out = np.concatenate([res.results[i]["out"] for i in range(NCORES)], axis=0)
    return out.reshape(B, C * K, H, W)



# revision 4
# speedup vs baseline: 1.5287x; 1.5287x over previous
"""ConnectionProductBlock on 8 TRN2 NeuronCores.

out[b, c*K + k, h, w] = am_out[b, c, h, w] * first_out[b, k, h, w]
  with B=16, C=8, K=64, H=W=56.

Data parallel over batch (2 batches per core, no communication). The whole
on-chip datapath is fp16 (the harness gate is L2 rel err < 2e-2; fp16
end-to-end lands at ~4e-4), which halves both HBM traffic and VectorE work
versus fp32:

  - SBUF layout: channels on partitions (p = b*64 + k), hw (=3136) on the
    free dim, so every DMA moves long contiguous runs.
  - am needs a partition-broadcast (am[b, c] replicated across the 64
    k-partitions of batch b). Compute engines have fixed lane<->partition
    wiring, so the fan-out runs on the idle TensorEngine: a K=16 selector
    matmul sel_c.T @ am writes rep[p, f] = am[p//64, c, f] into PSUM
    (fp32 - TRN2 matmul cannot write 16-bit PSUM).
  - PSUM fp32 poisons the VectorE fast path (a 32-bit operand drops
    tensor_tensor to 1x = 1 elem/lane/cyc), so each row is split:
      chunks 0-3: ScalarE evicts PSUM -> SBUF fp16 (otherwise-idle engine),
                  then ONE VectorE mul runs in 2x packed mode.
      chunks 4-6: VectorE multiplies straight from fp32 PSUM at 1x.
    This balances ScalarE (~16us) and VectorE (~22us) instead of putting
    ~34us of 1x multiplies on VectorE alone (the old fp32 design).
  - Output DMAs (fp16, one [128, 3136] transfer per c) alternate between
    the two HWDGE rings; the host reassembles channel order and upcasts.
"""

import numpy as np

B, C, K, H, W = 16, 8, 64, 56, 56
HW = H * W  # 3136
NCORES = 8
BPC = B // NCORES  # batches per core = 2
CH = 448  # 3136 = 7 * 448; fp32 PSUM chunk (one matmul, within one bank)
NFAST = 4  # chunks 0-3 -> scalar eviction + 2x mul; 4-6 -> 1x PSUM mul

_PROGRAMS = {}


def _build_program():
    import concourse.bacc as bacc
    import concourse.mybir as mybir
    import concourse.tile as tile

    nc = bacc.Bacc("TRN2", debug=False)
    # am rows (p = b*C + c) with the per-c [16, 128] selector blocks appended
    # on the free dim so one DMA covers data + selectors.
    amsel = nc.dram_tensor(
        "amsel", [BPC * C, HW + C * BPC * K], mybir.dt.float16, kind="ExternalInput"
    )
    first = nc.dram_tensor(
        "first", [BPC * K, HW], mybir.dt.float16, kind="ExternalInput"
    )
    out = nc.dram_tensor(
        "out", [C, BPC * K, HW], mybir.dt.float16, kind="ExternalOutput"
    )

    f32, f16 = mybir.dt.float32, mybir.dt.float16

    with tile.TileContext(nc) as tc:
        with (
            tc.tile_pool(name="ins", bufs=1) as ins_pool,
            # 2-bank fp32 psum tiles: chunk pairs at offsets 0 and 512.
            # tags: ps2 x3 bufs (6 banks) + ps1 x2 bufs (2 banks) = full PSUM.
            tc.tile_pool(name="ps2", bufs=3, space="PSUM") as ps2_pool,
            tc.tile_pool(name="ps1", bufs=2, space="PSUM") as ps1_pool,
            tc.tile_pool(name="rep", bufs=2) as rep_pool,
            tc.tile_pool(name="outs", bufs=3) as out_pool,
        ):
            first2 = ins_pool.tile([BPC * K, HW], f16)
            nc.sync.dma_start(out=first2[:], in_=first.ap())
            am = ins_pool.tile([BPC * C, HW + C * BPC * K], f16)
            nc.sync.dma_start(out=am[:], in_=amsel.ap())

            out_ap = out.ap()
            for c in range(C):
                sel = am[:, HW + c * BPC * K : HW + (c + 1) * BPC * K]
                # PE: rep[p, f] = am[p // 64, c, f] into fp32 PSUM.
                # 2-bank tiles, chunks at fp32 offsets 0 / 512 (bank-aligned).
                pts = []
                for t in range(3):  # chunks 0..5 in 2-bank tiles
                    pt = ps2_pool.tile([BPC * K, 1024], f32, tag="ps2")
                    for half in range(2):
                        j = 2 * t + half
                        nc.tensor.matmul(
                            pt[:, half * 512 : half * 512 + CH],
                            lhsT=sel,
                            rhs=am[:, j * CH : (j + 1) * CH],
                            start=True,
                            stop=True,
                        )
                    pts.append(pt)
                p1 = ps1_pool.tile([BPC * K, 512], f32, tag="ps1")
                nc.tensor.matmul(
                    p1[:, 0:CH],
                    lhsT=sel,
                    rhs=am[:, 6 * CH : 7 * CH],
                    start=True,
                    stop=True,
                )

                out_t = out_pool.tile([BPC * K, HW], f16, tag="out")
                # chunks 0-3: ScalarE evicts fp32 PSUM -> fp16 SBUF ...
                rep = rep_pool.tile([BPC * K, NFAST * CH], f16, tag="rep")
                for t in range(NFAST // 2):
                    nc.scalar.copy(
                        rep[:, t * 2 * CH : (t + 1) * 2 * CH].rearrange(
                            "p (u f) -> p u f", u=2
                        ),
                        pts[t][:].rearrange("p (u f) -> p u f", u=2)[:, :, 0:CH],
                    )
                # ... then one packed-2x fp16 mul over the whole fast span.
                nc.vector.tensor_mul(
                    out_t[:, 0 : NFAST * CH],
                    first2[:, 0 : NFAST * CH],
                    rep[:],
                )
                # chunks 4-5: 1x mul straight from the fp32 psum pair
                nc.vector.tensor_mul(
                    out_t[:, 4 * CH : 6 * CH].rearrange("p (u f) -> p u f", u=2),
                    first2[:, 4 * CH : 6 * CH].rearrange("p (u f) -> p u f", u=2),
                    pts[2][:].rearrange("p (u f) -> p u f", u=2)[:, :, 0:CH],
                )
                # chunk 6
                nc.vector.tensor_mul(
                    out_t[:, 6 * CH : 7 * CH],
                    first2[:, 6 * CH : 7 * CH],
                    p1[:, 0:CH],
                )
                # One [128, HW] fp16 DMA per c; alternate the two HWDGE rings.
                eng = nc.sync if c % 2 == 0 else nc.scalar
                eng.dma_start(out=out_ap[c], in_=out_t[:])
    nc.compile()
    return nc


def _get_program():
    if "v3" not in _PROGRAMS:
        _PROGRAMS["v3"] = _build_program()
    return _PROGRAMS["v3"]


def _make_sel():
    # One [16, 128] selector block per c: sel[b*C + c, c*128 + b*64 + k] = 1
    sel = np.zeros((BPC * C, C * BPC * K), dtype=np.float16)
    for c in range(C):
        for b in range(BPC):
            sel[b * C + c, c * BPC * K + b * K : c * BPC * K + (b + 1) * K] = 1.0
    return sel


_SEL = _make_sel()


def _make_amsel(am_core):
    """am_core [BPC*C, HW] fp32 -> [BPC*C, HW + 1024] fp16 with selector."""
    return np.ascontiguousarray(
        np.concatenate([am_core.astype(np.float16), _SEL], axis=1)
    )


def _run(am_np, first_np, **spmd_kwargs):
    from concourse.bass_utils import run_bass_kernel_spmd

    nc = _get_program()
    in_maps = []
    for i in range(NCORES):
        am_i = am_np[BPC * i : BPC * (i + 1)].reshape(BPC * C, HW)
        in_maps.append(
            {
                "amsel": _make_amsel(am_i),
                "first": np.ascontiguousarray(
                    first_np[BPC * i : BPC * (i + 1)]
                    .reshape(BPC * K, HW)
                    .astype(np.float16)
                ),
            }
        )
    return run_bass_kernel_spmd(
        nc, in_maps, core_ids=list(range(NCORES)), **spmd_kwargs
    )


def kernel(am_out, first_out):
    am_np = np.asarray(am_out, dtype=np.float32).reshape(B, C, HW)
    first_np = np.asarray(first_out, dtype=np.float32).reshape(B, K, HW)
    res = _run(am_np, first_np)
    # out[c, b*64 + k, f] -> full[b, c*64 + k, f], upcast to fp32
    parts = []
    for i in range(NCORES):
        o = res.results[i]["out"].reshape(C, BPC, K, HW)
        parts.append(np.transpose(o, (1, 0, 2, 3)).reshape(BPC, C * K, HW))
    out = np.concatenate(parts, axis=0).astype(np.float32)
    return out.reshape(B, C * K, H, W)


# revision 5
# speedup vs baseline: 1.6647x; 1.0890x over previous
"""ConnectionProductBlock on 8 TRN2 NeuronCores.

out[b, c*K + k, h, w] = am_out[b, c, h, w] * first_out[b, k, h, w]
  with B=16, C=8, K=64, H=W=56.

Data parallel over batch (2 batches per core, no communication). The whole
on-chip datapath is fp16 (the harness gate is L2 rel err < 2e-2; fp16
end-to-end lands at ~4e-4), which halves both HBM traffic and VectorE work
versus fp32:

  - SBUF layout: channels on partitions (p = b*64 + k), hw (=3136) on the
    free dim, so every DMA moves long contiguous runs.
  - am needs a partition-broadcast (am[b, c] replicated across the 64
    k-partitions of batch b). Compute engines have fixed lane<->partition
    wiring, so the fan-out runs on the idle TensorEngine: a K=16 selector
    matmul sel_c.T @ am writes rep[p, f] = am[p//64, c, f] into PSUM
    (fp32 - TRN2 matmul cannot write 16-bit PSUM).
  - PSUM fp32 poisons the VectorE fast path (a 32-bit operand drops
    tensor_tensor to 1x = 1 elem/lane/cyc), so each row is split:
      chunks 0-3: ScalarE evicts PSUM -> SBUF fp16 (otherwise-idle engine),
                  then ONE VectorE mul runs in 2x packed mode.
      chunks 4-6: VectorE multiplies straight from fp32 PSUM at 1x.
    This balances ScalarE (~16us) and VectorE (~22us) instead of putting
    ~34us of 1x multiplies on VectorE alone (the old fp32 design).
  - Output DMAs (fp16, one [128, 3136] transfer per c) alternate between
    the two HWDGE rings; the host reassembles channel order and upcasts.
"""

import numpy as np

B, C, K, H, W = 16, 8, 64, 56, 56
HW = H * W  # 3136
NCORES = 8
BPC = B // NCORES  # batches per core = 2
CH = 448  # 3136 = 7 * 448; fp32 PSUM chunk (one matmul, within one bank)
NFAST = 4  # chunks 0-3 -> scalar eviction + 2x mul; 4-6 -> 1x PSUM mul

_PROGRAMS = {}


def _build_program():
    import concourse.bacc as bacc
    import concourse.mybir as mybir
    import concourse.tile as tile

    nc = bacc.Bacc("TRN2", debug=False)
    # am rows (p = b*C + c) with the per-c [16, 128] selector blocks appended
    # on the free dim so one DMA covers data + selectors.
    amsel = nc.dram_tensor(
        "amsel", [BPC * C, HW + C * BPC * K], mybir.dt.float16, kind="ExternalInput"
    )
    first = nc.dram_tensor(
        "first", [BPC * K, HW], mybir.dt.float16, kind="ExternalInput"
    )
    out = nc.dram_tensor(
        "out", [C, BPC * K, HW], mybir.dt.float16, kind="ExternalOutput"
    )

    f32, f16 = mybir.dt.float32, mybir.dt.float16

    FAST = NFAST * CH  # 1792
    AMW = HW + C * BPC * K  # amsel row width

    with tile.TileContext(nc) as tc:
        with (
            tc.tile_pool(name="ins", bufs=1) as ins_pool,
            # fp32 psum: 2x 2-bank chunk-pair tiles + 1x 3-bank trio tile
            # = 7 banks in flight (2*4KB*2 + 6KB = 14KB of 16KB).
            tc.tile_pool(name="ps2", bufs=2, space="PSUM") as ps2_pool,
            tc.tile_pool(name="ps3", bufs=1, space="PSUM") as ps3_pool,
            tc.tile_pool(name="rep", bufs=2) as rep_pool,
            tc.tile_pool(name="outf", bufs=3) as outf_pool,
            tc.tile_pool(name="outs", bufs=3) as outs_pool,
        ):
            # amsel replicated at partition bases 0/32/64/96: K=16 selector
            # matmuls then row-tile into 4 concurrent 32-row PE groups.
            amt = ins_pool.tile([128, AMW], f16)
            for g in range(4):
                nc.scalar.dma_start(out=amt[32 * g : 32 * g + 16, :], in_=amsel.ap())
            # first in halves so the c=0 fast mul starts before the full load
            first2 = ins_pool.tile([BPC * K, HW], f16)
            nc.scalar.dma_start(out=first2[:, 0:FAST], in_=first.ap()[:, 0:FAST])
            nc.scalar.dma_start(out=first2[:, FAST:HW], in_=first.ap()[:, FAST:HW])

            out_ap = out.ap()
            for c in range(C):
                # PE: rep[p, f] = am[p // 64, c, f] into fp32 PSUM, one
                # matmul per 448-chunk, 4 (then 3) concurrent row-groups.
                def mm(dst, j):
                    g = j % 4
                    base = 32 * g
                    nc.tensor.matmul(
                        dst,
                        lhsT=amt[
                            base : base + BPC * C,
                            HW + c * BPC * K : HW + (c + 1) * BPC * K,
                        ],
                        rhs=amt[base : base + BPC * C, j * CH : (j + 1) * CH],
                        start=True,
                        stop=True,
                        tile_position=(base, 0),
                    )

                pts = []
                for t in range(2):  # chunks 0-3: pairs at offsets 0 / 512
                    pt = ps2_pool.tile([BPC * K, 1024], f32, tag="ps2")
                    mm(pt[:, 0:CH], 2 * t)
                    mm(pt[:, 512 : 512 + CH], 2 * t + 1)
                    pts.append(pt)
                trio = ps3_pool.tile([BPC * K, 1536], f32, tag="ps3")
                for u in range(3):  # chunks 4-6 at offsets 0 / 512 / 1024
                    mm(trio[:, u * 512 : u * 512 + CH], 4 + u)

                # chunks 0-3: ScalarE evicts fp32 PSUM -> fp16 SBUF ...
                rep = rep_pool.tile([BPC * K, FAST], f16, tag="rep")
                for t in range(2):
                    nc.scalar.copy(
                        rep[:, t * 2 * CH : (t + 1) * 2 * CH].rearrange(
                            "p (u f) -> p u f", u=2
                        ),
                        pts[t][:].rearrange("p (u f) -> p u f", u=2)[:, :, 0:CH],
                    )
                # ... then one packed-2x fp16 mul over the whole fast span
                out_f = outf_pool.tile([BPC * K, FAST], f16, tag="outf")
                nc.vector.tensor_mul(out_f[:], first2[:, 0:FAST], rep[:])
                nc.sync.dma_start(out=out_ap[c][:, 0:FAST], in_=out_f[:])

                # chunks 4-6: one 1x mul straight from the fp32 psum trio
                out_s = outs_pool.tile([BPC * K, HW - FAST], f16, tag="outs")
                nc.vector.tensor_mul(
                    out_s[:].rearrange("p (u f) -> p u f", u=3),
                    first2[:, FAST:HW].rearrange("p (u f) -> p u f", u=3),
                    trio[:].rearrange("p (u f) -> p u f", u=3)[:, :, 0:CH],
                )
                nc.gpsimd.dma_start(out=out_ap[c][:, FAST:HW], in_=out_s[:])
    nc.compile()
    return nc


def _get_program():
    if "v3" not in _PROGRAMS:
        _PROGRAMS["v3"] = _build_program()
    return _PROGRAMS["v3"]


def _make_sel():
    # One [16, 128] selector block per c: sel[b*C + c, c*128 + b*64 + k] = 1
    sel = np.zeros((BPC * C, C * BPC * K), dtype=np.float16)
    for c in range(C):
        for b in range(BPC):
            sel[b * C + c, c * BPC * K + b * K : c * BPC * K + (b + 1) * K] = 1.0
    return sel


_SEL = _make_sel()


def _make_amsel(am_core):
    """am_core [BPC*C, HW] fp32 -> [BPC*C, HW + 1024] fp16 with selector."""
    return np.ascontiguousarray(
        np.concatenate([am_core.astype(np.float16), _SEL], axis=1)
    )


def _run(am_np, first_np, **spmd_kwargs):
    from concourse.bass_utils import run_bass_kernel_spmd

    nc = _get_program()
    in_maps = []
    for i in range(NCORES):
        am_i = am_np[BPC * i : BPC * (i + 1)].reshape(BPC * C, HW)
        in_maps.append(
            {
                "amsel": _make_amsel(am_i),
                "first": np.ascontiguousarray(
                    first_np[BPC * i : BPC * (i + 1)]
                    .reshape(BPC * K, HW)
                    .astype(np.float16)
                ),
            }
        )
    return run_bass_kernel_spmd(
        nc, in_maps, core_ids=list(range(NCORES)), **spmd_kwargs
    )


def kernel(am_out, first_out):
    am_np = np.asarray(am_out, dtype=np.float32).reshape(B, C, HW)
    first_np = np.asarray(first_out, dtype=np.float32).reshape(B, K, HW)
    res = _run(am_np, first_np)
    # out[c, b*64 + k, f] -> full[b, c*64 + k, f], upcast to fp32
    parts = []
    for i in range(NCORES):
        o = res.results[i]["out"].reshape(C, BPC, K, HW)
        parts.append(np.transpose(o, (1, 0, 2, 3)).reshape(BPC, C * K, HW))
    out = np.concatenate(parts, axis=0).astype(np.float32)
    return out.reshape(B, C * K, H, W)
